# revision 10
# baseline (speedup 1.0000x reference)
"""Trainium2 Bass kernel for nn_ConditionalMoELayer.

Strategy (data-parallel over tokens, dense-expert compute on device):
  - 8192 tokens are split across 8 NeuronCores (1024 tokens each).
  - Each core computes the full layer for its tokens: difficulty net,
    gating, per-token top-k softmax routing weights (dense_w), and the
    weighted combination of all 4 expert FFNs.
  - Routing matmuls (difficulty + gate) run in true fp32 (4-pass PE) so the
    discrete k / top-k decisions match the fp32 reference; the heavy expert
    FFN matmuls run in fp32r (single pass, full PE rate).
  - Weights are loaded from HBM exactly once per core (~67 MB, overlapped
    with ~440us of PE work), tokens/activations stay SBUF-resident.
  - Host only does layout transforms (transpose/tiling) and the final
    concat; the optional eb2 bias term (zero in practice) is applied on
    host from the device-computed dense_w.
"""

import numpy as np

P = 128          # partitions
D = 1024         # d_model
H = 2048         # expert hidden
E = 4            # experts
H1 = 512         # difficulty-net hidden
T_FULL = 8192    # total tokens
N_CORES = 8
TC = T_FULL // N_CORES   # tokens per core = 1024

KD = D // P      # 8   k-subtiles over D
KH = H // P      # 16  k-subtiles over H
MH1 = H1 // P    # 4   m-tiles of difficulty hidden
NT = TC // P     # 8   token tiles per core
DC = 256         # output d chunk
NDC = D // DC    # 4
TCH = 512        # token chunk for first-layer matmuls
NTCH = TC // TCH # 2

TH_LO = 0.5
TH_HI = 2.0
MIN_E = 1


def build_nc():
    """Build the Bass module (single SPMD program, same on all 8 cores)."""
    import concourse.mybir as mybir
    import concourse.tile as tile
    from concourse import bacc

    f32 = mybir.dt.float32
    f32r = mybir.dt.float32r
    AF = mybir.ActivationFunctionType
    OP = mybir.AluOpType

    nc = bacc.Bacc(None, target_bir_lowering=False, debug=False)

    with tile.TileContext(nc) as tc:
        from contextlib import ExitStack

        with ExitStack() as ctx:
            dram = ctx.enter_context(tc.tile_pool(name="dram", bufs=1, space="DRAM"))
            # ---- external I/O ----
            xt_d = dram.tile([P, KD, TC], f32r, kind="ExternalInput", name="xt")
            w1t_d = dram.tile([E, KH, P, KD, P], f32r, kind="ExternalInput", name="w1t")
            w2t_d = dram.tile([E, NDC, P, KH, DC], f32r, kind="ExternalInput", name="w2t")
            dp1t_d = dram.tile([P, KD, H1], f32, kind="ExternalInput", name="dp1t")
            dp2t_d = dram.tile([P, MH1], f32, kind="ExternalInput", name="dp2t")
            gwt_d = dram.tile([P, KD, E], f32, kind="ExternalInput", name="gwt")
            gbt_d = dram.tile([P, E], f32, kind="ExternalInput", name="gbt")
            dpb1_d = dram.tile([P, MH1], f32, kind="ExternalInput", name="dpb1")
            dpb2_d = dram.tile([P, 1], f32, kind="ExternalInput", name="dpb2")
            eb1t_d = dram.tile([E, P, KH], f32, kind="ExternalInput", name="eb1t")
            y_d = dram.tile([P, NT, D], f32, kind="ExternalOutput", name="y")
            dw_d = dram.tile([P, NT, E], f32, kind="ExternalOutput", name="dw")

            # ---- SBUF pools ----
            const = ctx.enter_context(tc.tile_pool(name="const", bufs=1))
            big = ctx.enter_context(tc.tile_pool(name="big", bufs=1))
            h1p = ctx.enter_context(tc.tile_pool(name="h1p", bufs=1))
            w1p = ctx.enter_context(tc.tile_pool(name="w1p", bufs=2))
            w2p = ctx.enter_context(tc.tile_pool(name="w2p", bufs=2))
            hidp = ctx.enter_context(tc.tile_pool(name="hidp", bufs=1))
            etp = ctx.enter_context(tc.tile_pool(name="etp", bufs=3))
            rp = ctx.enter_context(tc.tile_pool(name="rp", bufs=1))
            # ---- PSUM pools ----
            psb = ctx.enter_context(tc.tile_pool(name="psb", bufs=3, space="PSUM"))
            ps2p = ctx.enter_context(tc.tile_pool(name="ps2p", bufs=2, space="PSUM"))
            pslp = ctx.enter_context(tc.tile_pool(name="pslp", bufs=2, space="PSUM"))
            psep = ctx.enter_context(tc.tile_pool(name="psep", bufs=1, space="PSUM"))

            # ---- persistent SBUF tensors ----
            xt_sb = big.tile([P, KD, TC], f32r, tag="xt")
            y_sb = big.tile([P, NT, D], f32, tag="y")
            dp1t_sb = const.tile([P, KD, H1], f32, tag="dp1t")
            dp2t_sb = const.tile([P, MH1], f32, tag="dp2t")
            gwt_sb = const.tile([P, KD, E], f32, tag="gwt")
            gbt_sb = const.tile([P, E], f32, tag="gbt")
            dpb1_sb = const.tile([P, MH1], f32, tag="dpb1")
            dpb2_sb = const.tile([P, 1], f32, tag="dpb2")
            eb1_sb = const.tile([P, E, KH], f32, tag="eb1")

            nc.sync.dma_start(xt_sb[:], xt_d[:])
            nc.sync.dma_start(dp1t_sb[:], dp1t_d[:])
            nc.sync.dma_start(dp2t_sb[:], dp2t_d[:])
            nc.sync.dma_start(gwt_sb[:], gwt_d[:])
            nc.sync.dma_start(gbt_sb[:], gbt_d[:])
            nc.sync.dma_start(dpb1_sb[:], dpb1_d[:])
            nc.sync.dma_start(dpb2_sb[:], dpb2_d[:])
            for e in range(E):
                nc.sync.dma_start(eb1_sb[:, e, :], eb1t_d[e])

            # routing scratch
            z_all = rp.tile([P, NT], f32, tag="zdp")
            L_all = rp.tile([P, NT, E], f32, tag="L")
            kk = rp.tile([P, NT], f32, tag="kk")
            tmp = rp.tile([P, NT], f32, tag="tmpr")
            rank = rp.tile([P, NT, E], f32, tag="rank")
            sel = rp.tile([P, NT, E], f32, tag="sel")
            mx = rp.tile([P, NT], f32, tag="mx")
            eL = rp.tile([P, NT, E], f32, tag="eL")
            den = rp.tile([P, NT], f32, tag="den")
            w_all = rp.tile([P, NT, E], f32, tag="w_all")

            # =========== Phase 1: routing (true fp32 matmuls) ===========
            for t2 in range(NTCH):
                tsl = slice(t2 * TCH, (t2 + 1) * TCH)
                h1T = h1p.tile([P, MH1, TCH], f32, tag="h1T")
                for mt in range(MH1):
                    ps = psb.tile([P, TCH], f32, tag="psb")
                    for ko in range(KD):
                        nc.tensor.matmul(
                            ps,
                            dp1t_sb[:, ko, mt * P:(mt + 1) * P],
                            xt_sb[:, ko, tsl].bitcast(f32),
                            start=(ko == 0),
                            stop=(ko == KD - 1),
                        )
                    nc.scalar.activation(
                        h1T[:, mt, :], ps, AF.Relu, bias=dpb1_sb[:, mt:mt + 1]
                    )
                for t4 in range(TCH // P):
                    ts = t2 * (TCH // P) + t4
                    t4sl = slice(t4 * P, (t4 + 1) * P)
                    pse = psep.tile([P, 1], f32, tag="pse")
                    for ko in range(MH1):
                        nc.tensor.matmul(
                            pse,
                            h1T[:, ko, t4sl],
                            dp2t_sb[:, ko:ko + 1],
                            start=(ko == 0),
                            stop=(ko == MH1 - 1),
                        )
                    nc.scalar.activation(
                        z_all[:, ts:ts + 1], pse, AF.Identity,
                        bias=dpb2_sb[:, 0:1],
                    )
                    psl = pslp.tile([P, E], f32, tag="psl")
                    for ko in range(KD):
                        nc.tensor.matmul(
                            psl,
                            xt_sb[:, ko, ts * P:(ts + 1) * P].bitcast(f32),
                            gwt_sb[:, ko, :],
                            start=(ko == 0),
                            stop=(ko == KD - 1),
                        )
                    nc.vector.tensor_add(L_all[:, ts, :], psl, gbt_sb[:])

            # ---- k per token (exact round-half-to-even semantics) ----
            # k = round(1 + 3*clip((softplus(z)-0.5)/1.5, 0, 1)); since
            # softplus is monotone, compare z against pre-inverted thresholds:
            # k-1 = [z >= zt1] + [z > zt2] + [z >= zt3]
            # where zt_i = log(expm1(TH_LO + (TH_HI-TH_LO)*(2i-1)/(2(E-1)))).
            # (zt2 strict: RNE rounds the kraw=2.5 midpoint down to k=2;
            # the 1.5 -> 2 and 3.5 -> 4 midpoints round up, matching >=.)
            step = (TH_HI - TH_LO) / (2 * (E - MIN_E))
            zts = [
                float(np.float32(np.log(np.expm1(TH_LO + (2 * i - 1) * step))))
                for i in (1, 2, 3)
            ]
            nc.vector.tensor_scalar(kk[:], z_all[:], zts[0], None, OP.is_ge)
            nc.vector.tensor_scalar(tmp[:], z_all[:], zts[1], None, OP.is_gt)
            nc.vector.tensor_add(kk[:], kk[:], tmp[:])
            nc.vector.tensor_scalar(tmp[:], z_all[:], zts[2], None, OP.is_ge)
            nc.vector.tensor_add(kk[:], kk[:], tmp[:])

            # ---- rank of each expert (0 = largest logit; ties -> lower idx) ----
            for i in range(E):
                acc = rank[:, :, i]
                first = True
                for j in range(E):
                    if j == i:
                        continue
                    if first:
                        nc.vector.tensor_tensor(
                            acc, L_all[:, :, j], L_all[:, :, i], op=OP.is_gt
                        )
                        first = False
                    else:
                        nc.vector.tensor_tensor(
                            tmp[:], L_all[:, :, j], L_all[:, :, i], op=OP.is_gt
                        )
                        nc.vector.tensor_add(acc, acc, tmp[:])
                    if j < i:
                        nc.vector.tensor_tensor(
                            tmp[:], L_all[:, :, j], L_all[:, :, i], op=OP.is_equal
                        )
                        nc.vector.tensor_add(acc, acc, tmp[:])

            # sel_i = rank_i <= k-1
            for i in range(E):
                nc.vector.tensor_tensor(
                    sel[:, :, i], rank[:, :, i], kk[:], op=OP.is_le
                )

            # ---- masked softmax over selected experts ----
            nc.vector.tensor_reduce(
                mx[:], L_all[:], axis=mybir.AxisListType.X, op=OP.max
            )
            nc.vector.tensor_tensor(
                eL[:], L_all[:],
                mx[:].unsqueeze(2).to_broadcast([P, NT, E]),
                op=OP.subtract,
            )
            nc.scalar.activation(eL[:], eL[:], AF.Exp)
            nc.vector.tensor_mul(eL[:], eL[:], sel[:])
            nc.vector.tensor_reduce(
                den[:], eL[:], axis=mybir.AxisListType.X, op=OP.add
            )
            nc.vector.reciprocal(den[:], den[:])
            nc.vector.tensor_tensor(
                w_all[:], eL[:],
                den[:].unsqueeze(2).to_broadcast([P, NT, E]),
                op=OP.mult,
            )
            nc.sync.dma_start(dw_d[:], w_all[:])

            # =========== Phase 2: expert FFNs (fp32r matmuls) ===========
            for e in range(E):
                hidT = hidp.tile([P, KH, TC], f32r, tag="hidT")
                for ht in range(KH):
                    w1s = w1p.tile([P, KD, P], f32r, tag="w1")
                    nc.sync.dma_start(w1s[:], w1t_d[e, ht])
                    for t2 in range(NTCH):
                        tsl = slice(t2 * TCH, (t2 + 1) * TCH)
                        ps = psb.tile([P, TCH], f32, tag="psb")
                        for ko in range(KD):
                            nc.tensor.matmul(
                                ps,
                                w1s[:, ko, :],
                                xt_sb[:, ko, tsl],
                                start=(ko == 0),
                                stop=(ko == KD - 1),
                            )
                        nc.scalar.activation(
                            hidT[:, ht, tsl], ps, AF.Relu,
                            bias=eb1_sb[:, e, ht:ht + 1],
                        )
                for dc in range(NDC):
                    dsl = slice(dc * DC, (dc + 1) * DC)
                    w2s = w2p.tile([P, KH, DC], f32r, tag="w2")
                    nc.sync.dma_start(w2s[:], w2t_d[e, dc])
                    for ts in range(NT):
                        ps2 = ps2p.tile([P, DC], f32, tag="ps2")
                        for ko in range(KH):
                            nc.tensor.matmul(
                                ps2,
                                hidT[:, ko, ts * P:(ts + 1) * P],
                                w2s[:, ko, :],
                                start=(ko == 0),
                                stop=(ko == KH - 1),
                            )
                        wsl = w_all[:, ts, e:e + 1]
                        ysl = y_sb[:, ts, dsl]
                        if e == 0:
                            nc.scalar.activation(ysl, ps2, AF.Copy, scale=wsl)
                        else:
                            t2b = etp.tile([P, DC], f32, tag="etmp")
                            nc.scalar.activation(t2b[:], ps2, AF.Copy, scale=wsl)
                            nc.vector.tensor_add(ysl, ysl, t2b[:])
                        if e == E - 1:
                            nc.sync.dma_start(y_d[:, ts, dsl], ysl)

    nc.compile()
    names = {
        "xt": xt_d, "w1t": w1t_d, "w2t": w2t_d, "dp1t": dp1t_d,
        "dp2t": dp2t_d, "gwt": gwt_d, "gbt": gbt_d, "dpb1": dpb1_d,
        "dpb2": dpb2_d, "eb1t": eb1t_d, "y": y_d, "dw": dw_d,
    }
    names = {k: _ap_name(v) for k, v in names.items()}
    return nc, names


def _ap_name(t):
    return t.tensor.name if hasattr(t, "tensor") else t.name


def prep_in_maps(x, gate_w, gate_b, dp_w1, dp_b1, dp_w2, dp_b2, ew1, eb1, ew2, eb2):
    """Host-side sharding: tile/transpose inputs into per-core input maps."""
    f32 = np.float32
    xf = np.ascontiguousarray(x.reshape(T_FULL, D).astype(f32, copy=False))

    w1t = np.ascontiguousarray(
        ew1.reshape(E, KD, P, KH, P).transpose(0, 3, 2, 1, 4)
    )  # [E, ht, p, ko, h]
    w2t = np.ascontiguousarray(
        ew2.reshape(E, KH, P, NDC, DC).transpose(0, 3, 2, 1, 4)
    )  # [E, dc, p, ko, d]
    dp1t = np.ascontiguousarray(dp_w1.reshape(KD, P, H1).transpose(1, 0, 2))
    dp2t = np.ascontiguousarray(dp_w2[:, 0].reshape(MH1, P).T)
    gwt = np.ascontiguousarray(gate_w.reshape(KD, P, E).transpose(1, 0, 2))
    gbt = np.ascontiguousarray(np.broadcast_to(gate_b, (P, E)))
    dpb1 = np.ascontiguousarray(dp_b1.reshape(MH1, P).T)
    dpb2 = np.full((P, 1), dp_b2[0], dtype=f32)
    eb1t = np.ascontiguousarray(eb1.reshape(E, KH, P).transpose(0, 2, 1))

    shared = {
        "w1t": w1t, "w2t": w2t, "dp1t": dp1t, "dp2t": dp2t,
        "gwt": gwt, "gbt": gbt, "dpb1": dpb1, "dpb2": dpb2, "eb1t": eb1t,
    }
    in_maps = []
    for c in range(N_CORES):
        xc = xf[c * TC:(c + 1) * TC]                       # [TC, D]
        xt = np.ascontiguousarray(
            xc.T.reshape(KD, P, TC).transpose(1, 0, 2)
        )                                                  # [p, ko, t]
        in_maps.append({"xt": xt, **shared})
    return in_maps


def remap_names(in_maps, names):
    return [{names[k]: v for k, v in m.items()} for m in in_maps]


def assemble(results, names, eb2):
    """Host-side unshard: concat per-core outputs; apply eb2 term if nonzero."""
    outs = []
    for c in range(N_CORES):
        y = results[c][names["y"]]                         # [P, NT, D]
        yc = np.ascontiguousarray(y.transpose(1, 0, 2)).reshape(TC, D)
        if np.any(eb2):
            dw = results[c][names["dw"]].transpose(1, 0, 2).reshape(TC, E)
            yc = yc + dw.astype(np.float64) @ eb2.astype(np.float64)
        outs.append(yc)
    out = np.concatenate(outs, axis=0).astype(np.float32)
    return out.reshape(4, 2048, D)


_BUILT = {}


def kernel(x, gate_w, gate_b, dp_w1, dp_b1, dp_w2, dp_b2, ew1, eb1, ew2, eb2,
           trace=False):
    from concourse.bass_utils import run_bass_kernel_spmd

    if "nc" not in _BUILT:
        _BUILT["nc"] = build_nc()
    nc, names = _BUILT["nc"]

    in_maps = remap_names(
        prep_in_maps(
            x, gate_w, gate_b, dp_w1, dp_b1, dp_w2, dp_b2, ew1, eb1, ew2, eb2
        ),
        names,
    )
    res = run_bass_kernel_spmd(nc, in_maps, list(range(N_CORES)), trace=trace)
    out = assemble(res.results, names, eb2)
    if trace:
        return out, res
    return out


# revision 12
# speedup vs baseline: 1.6558x; 1.6558x over previous
"""Trainium2 Bass kernel for nn_ConditionalMoELayer (expert-parallel, sparse).

Two-phase design following the expert-parallel sharding hint:

Phase 1 (routing, data-parallel): the 8192 tokens are split across the 8
cores (1024 each). Each core runs the difficulty net, the gate, and the
per-token top-k softmax on its tokens, producing dense routing weights
dense_w[t, e]. Routing matmuls run in true fp32 (4-pass PE) so the discrete
k / top-k decisions match the fp32 reference exactly.

Host dispatch (the "all-to-all"): tokens are grouped by assigned expert
(dense_w[t,e] > 0); each expert's token set is split over 2 cores
(4 experts x 2 = 8 cores), zero-padded to a common static capacity C.

Phase 2 (expert FFN, expert-parallel): each core holds ONE expert's weights
and computes y = relu(x @ W1 + b1) @ W2 for its gathered tokens in fp32r
(single-pass PE, full rate). All matmuls have 512-wide moving operands so
the 4-byte weight loads stay hidden. Host scales rows by dense_w and
scatter-adds into the output (token sets within one expert are disjoint).

Only ~40% of token-expert pairs are active, so phase 2 does ~2.5x fewer
FLOPs than the dense equivalent.
"""

import numpy as np

P = 128          # partitions
D = 1024         # d_model
H = 2048         # expert hidden
E = 4            # experts
H1 = 512         # difficulty-net hidden
T_FULL = 8192    # total tokens
N_CORES = 8
TC = T_FULL // N_CORES   # tokens per core in phase 1

KD = D // P      # 8   k-subtiles over D
KH = H // P      # 16  k-subtiles over H
MH1 = H1 // P    # 4   m-tiles of difficulty hidden
NT = TC // P     # 8   token tiles per core (phase 1)
ND = D // P      # 8   d-tiles (phase 2 output)
TCH = 512        # token chunk for matmul moving operand
NTCH = TC // TCH # 2

TH_LO = 0.5
TH_HI = 2.0
MIN_E = 1


def _ap_name(t):
    return t.tensor.name if hasattr(t, "tensor") else t.name


def build_routing_nc():
    """Phase-1 module: difficulty net + gate + top-k softmax -> dense_w."""
    import concourse.mybir as mybir
    import concourse.tile as tile
    from concourse import bacc
    from contextlib import ExitStack

    f32 = mybir.dt.float32
    AF = mybir.ActivationFunctionType
    OP = mybir.AluOpType

    nc = bacc.Bacc(None, target_bir_lowering=False, debug=False)

    with tile.TileContext(nc) as tc:
        with ExitStack() as ctx:
            dram = ctx.enter_context(tc.tile_pool(name="dram", bufs=1, space="DRAM"))
            xt_d = dram.tile([P, KD, TC], f32, kind="ExternalInput", name="xt")
            dp1t_d = dram.tile([P, KD, H1], f32, kind="ExternalInput", name="dp1t")
            dp2t_d = dram.tile([P, MH1], f32, kind="ExternalInput", name="dp2t")
            gwt_d = dram.tile([P, KD, E], f32, kind="ExternalInput", name="gwt")
            gbt_d = dram.tile([P, E], f32, kind="ExternalInput", name="gbt")
            dpb1_d = dram.tile([P, MH1], f32, kind="ExternalInput", name="dpb1")
            dpb2_d = dram.tile([P, 1], f32, kind="ExternalInput", name="dpb2")
            dw_d = dram.tile([P, NT, E], f32, kind="ExternalOutput", name="dw")

            const = ctx.enter_context(tc.tile_pool(name="const", bufs=1))
            big = ctx.enter_context(tc.tile_pool(name="big", bufs=1))
            h1p = ctx.enter_context(tc.tile_pool(name="h1p", bufs=2))
            rp = ctx.enter_context(tc.tile_pool(name="rp", bufs=1))
            psb = ctx.enter_context(tc.tile_pool(name="psb", bufs=4, space="PSUM"))
            pslp = ctx.enter_context(tc.tile_pool(name="pslp", bufs=2, space="PSUM"))
            psep = ctx.enter_context(tc.tile_pool(name="psep", bufs=2, space="PSUM"))

            xt_sb = big.tile([P, KD, TC], f32, tag="xt")
            dp1t_sb = const.tile([P, KD, H1], f32, tag="dp1t")
            dp2t_sb = const.tile([P, MH1], f32, tag="dp2t")
            gwt_sb = const.tile([P, KD, E], f32, tag="gwt")
            gbt_sb = const.tile([P, E], f32, tag="gbt")
            dpb1_sb = const.tile([P, MH1], f32, tag="dpb1")
            dpb2_sb = const.tile([P, 1], f32, tag="dpb2")

            nc.sync.dma_start(dp1t_sb[:], dp1t_d[:])
            nc.sync.dma_start(dp2t_sb[:], dp2t_d[:])
            nc.sync.dma_start(gwt_sb[:], gwt_d[:])
            nc.sync.dma_start(gbt_sb[:], gbt_d[:])
            nc.sync.dma_start(dpb1_sb[:], dpb1_d[:])
            nc.sync.dma_start(dpb2_sb[:], dpb2_d[:])
            # chunked x load so compute starts as soon as chunk 0 lands
            for t2 in range(NTCH):
                nc.sync.dma_start(
                    xt_sb[:, :, t2 * TCH:(t2 + 1) * TCH],
                    xt_d[:, :, t2 * TCH:(t2 + 1) * TCH],
                )

            z_all = rp.tile([P, NT], f32, tag="zdp")
            L_all = rp.tile([P, NT, E], f32, tag="L")
            kk = rp.tile([P, NT], f32, tag="kk")
            tmp = rp.tile([P, NT], f32, tag="tmpr")
            rank = rp.tile([P, NT, E], f32, tag="rank")
            sel = rp.tile([P, NT, E], f32, tag="sel")
            mx = rp.tile([P, NT], f32, tag="mx")
            eL = rp.tile([P, NT, E], f32, tag="eL")
            den = rp.tile([P, NT], f32, tag="den")
            w_all = rp.tile([P, NT, E], f32, tag="w_all")

            for t2 in range(NTCH):
                tsl = slice(t2 * TCH, (t2 + 1) * TCH)
                h1T = h1p.tile([P, MH1, TCH], f32, tag="h1T")
                for mt in range(MH1):
                    ps = psb.tile([P, TCH], f32, tag="psb")
                    for ko in range(KD):
                        nc.tensor.matmul(
                            ps,
                            dp1t_sb[:, ko, mt * P:(mt + 1) * P],
                            xt_sb[:, ko, tsl],
                            start=(ko == 0),
                            stop=(ko == KD - 1),
                        )
                    nc.scalar.activation(
                        h1T[:, mt, :], ps, AF.Relu, bias=dpb1_sb[:, mt:mt + 1]
                    )
                for t4 in range(TCH // P):
                    ts = t2 * (TCH // P) + t4
                    t4sl = slice(t4 * P, (t4 + 1) * P)
                    pse = psep.tile([P, 1], f32, tag="pse")
                    for ko in range(MH1):
                        nc.tensor.matmul(
                            pse,
                            h1T[:, ko, t4sl],
                            dp2t_sb[:, ko:ko + 1],
                            start=(ko == 0),
                            stop=(ko == MH1 - 1),
                        )
                    nc.scalar.activation(
                        z_all[:, ts:ts + 1], pse, AF.Identity,
                        bias=dpb2_sb[:, 0:1],
                    )
                    psl = pslp.tile([P, E], f32, tag="psl")
                    for ko in range(KD):
                        nc.tensor.matmul(
                            psl,
                            xt_sb[:, ko, ts * P:(ts + 1) * P],
                            gwt_sb[:, ko, :],
                            start=(ko == 0),
                            stop=(ko == KD - 1),
                        )
                    nc.vector.tensor_add(L_all[:, ts, :], psl, gbt_sb[:])

            # k = round(1 + 3*clip((softplus(z)-0.5)/1.5, 0, 1)) via monotone
            # z-space thresholds; zt2 strict (RNE rounds kraw=2.5 down).
            step = (TH_HI - TH_LO) / (2 * (E - MIN_E))
            zts = [
                float(np.float32(np.log(np.expm1(TH_LO + (2 * i - 1) * step))))
                for i in (1, 2, 3)
            ]
            nc.vector.tensor_scalar(kk[:], z_all[:], zts[0], None, OP.is_ge)
            nc.vector.tensor_scalar(tmp[:], z_all[:], zts[1], None, OP.is_gt)
            nc.vector.tensor_add(kk[:], kk[:], tmp[:])
            nc.vector.tensor_scalar(tmp[:], z_all[:], zts[2], None, OP.is_ge)
            nc.vector.tensor_add(kk[:], kk[:], tmp[:])

            # rank of each expert (0 = largest logit; ties -> lower index)
            for i in range(E):
                acc = rank[:, :, i]
                first = True
                for j in range(E):
                    if j == i:
                        continue
                    if first:
                        nc.vector.tensor_tensor(
                            acc, L_all[:, :, j], L_all[:, :, i], op=OP.is_gt
                        )
                        first = False
                    else:
                        nc.vector.tensor_tensor(
                            tmp[:], L_all[:, :, j], L_all[:, :, i], op=OP.is_gt
                        )
                        nc.vector.tensor_add(acc, acc, tmp[:])
                    if j < i:
                        nc.vector.tensor_tensor(
                            tmp[:], L_all[:, :, j], L_all[:, :, i], op=OP.is_equal
                        )
                        nc.vector.tensor_add(acc, acc, tmp[:])

            for i in range(E):
                nc.vector.tensor_tensor(
                    sel[:, :, i], rank[:, :, i], kk[:], op=OP.is_le
                )

            nc.vector.tensor_reduce(
                mx[:], L_all[:], axis=mybir.AxisListType.X, op=OP.max
            )
            nc.vector.tensor_tensor(
                eL[:], L_all[:],
                mx[:].unsqueeze(2).to_broadcast([P, NT, E]),
                op=OP.subtract,
            )
            nc.scalar.activation(eL[:], eL[:], AF.Exp)
            nc.vector.tensor_mul(eL[:], eL[:], sel[:])
            nc.vector.tensor_reduce(
                den[:], eL[:], axis=mybir.AxisListType.X, op=OP.add
            )
            nc.vector.reciprocal(den[:], den[:])
            nc.vector.tensor_tensor(
                w_all[:], eL[:],
                den[:].unsqueeze(2).to_broadcast([P, NT, E]),
                op=OP.mult,
            )
            nc.sync.dma_start(dw_d[:], w_all[:])

    nc.compile()
    names = {k: _ap_name(v) for k, v in {
        "xt": xt_d, "dp1t": dp1t_d, "dp2t": dp2t_d, "gwt": gwt_d,
        "gbt": gbt_d, "dpb1": dpb1_d, "dpb2": dpb2_d, "dw": dw_d,
    }.items()}
    return nc, names


def build_ffn_nc(C):
    """Phase-2 module: one expert FFN over C gathered tokens per core.

    All matmuls use up-to-512-wide moving operands (tokens): mm1 computes
    hid^T = W1^T x^T, mm2 computes y^T = W2^T hid^T (weights stationary).
    """
    import concourse.mybir as mybir
    import concourse.tile as tile
    from concourse import bacc
    from contextlib import ExitStack

    f32 = mybir.dt.float32
    f32r = mybir.dt.float32r
    AF = mybir.ActivationFunctionType

    chunks = []
    c0 = 0
    while c0 < C:
        cw = min(TCH, C - c0)
        chunks.append((c0, cw))
        c0 += cw

    nc = bacc.Bacc(None, target_bir_lowering=False, debug=False)

    with tile.TileContext(nc) as tc:
        with ExitStack() as ctx:
            dram = ctx.enter_context(tc.tile_pool(name="dram", bufs=1, space="DRAM"))
            xg_d = dram.tile([P, KD, C], f32r, kind="ExternalInput", name="xg")
            w1_d = dram.tile([P, KH, KD, P], f32r, kind="ExternalInput", name="w1g")
            w2_d = dram.tile([ND, P, KH, P], f32r, kind="ExternalInput", name="w2g")
            eb1_d = dram.tile([P, KH], f32, kind="ExternalInput", name="eb1g")
            yt_d = dram.tile([P, ND, C], f32, kind="ExternalOutput", name="yt")

            const = ctx.enter_context(tc.tile_pool(name="const", bufs=1))
            xgp = ctx.enter_context(tc.tile_pool(name="xgp", bufs=2))
            hidp = ctx.enter_context(tc.tile_pool(name="hidp", bufs=1))
            w2p = ctx.enter_context(tc.tile_pool(name="w2p", bufs=3))
            ytp = ctx.enter_context(tc.tile_pool(name="ytp", bufs=2))
            psb = ctx.enter_context(tc.tile_pool(name="psb", bufs=3, space="PSUM"))
            ps2p = ctx.enter_context(tc.tile_pool(name="ps2p", bufs=3, space="PSUM"))

            w1_sb = const.tile([P, KH, KD, P], f32r, tag="w1")
            eb1_sb = const.tile([P, KH], f32, tag="eb1")
            nc.sync.dma_start(eb1_sb[:], eb1_d[:])
            # W1 resident (64KB/partition), loaded per-ht so matmuls can
            # start before the whole 8MB lands
            for ht in range(KH):
                nc.sync.dma_start(w1_sb[:, ht], w1_d[:, ht])

            for (c0, cw) in chunks:
                csl = slice(c0, c0 + cw)
                xg_sb = xgp.tile([P, KD, TCH], f32r, tag="xg")
                nc.sync.dma_start(xg_sb[:, :, :cw], xg_d[:, :, csl])
                hidT = hidp.tile([P, KH, TCH], f32r, tag="hid")
                for ht in range(KH):
                    ps = psb.tile([P, TCH], f32, tag="psb")
                    for ko in range(KD):
                        nc.tensor.matmul(
                            ps[:, :cw],
                            w1_sb[:, ht, ko, :],
                            xg_sb[:, ko, :cw],
                            start=(ko == 0),
                            stop=(ko == KD - 1),
                        )
                    nc.scalar.activation(
                        hidT[:, ht, :cw], ps[:, :cw], AF.Relu,
                        bias=eb1_sb[:, ht:ht + 1],
                    )
                yt_sb = ytp.tile([P, ND, TCH], f32, tag="yt")
                for dt in range(ND):
                    w2s = w2p.tile([P, KH, P], f32r, tag="w2")
                    nc.sync.dma_start(w2s[:], w2_d[dt])
                    ps2 = ps2p.tile([P, TCH], f32, tag="ps2")
                    for ko in range(KH):
                        nc.tensor.matmul(
                            ps2[:, :cw],
                            w2s[:, ko, :],
                            hidT[:, ko, :cw],
                            start=(ko == 0),
                            stop=(ko == KH - 1),
                        )
                    nc.vector.tensor_copy(yt_sb[:, dt, :cw], ps2[:, :cw])
                nc.sync.dma_start(yt_d[:, :, csl], yt_sb[:, :, :cw])

    nc.compile()
    names = {k: _ap_name(v) for k, v in {
        "xg": xg_d, "w1g": w1_d, "w2g": w2_d, "eb1g": eb1_d, "yt": yt_d,
    }.items()}
    return nc, names


def prep_routing_in_maps(x, gate_w, gate_b, dp_w1, dp_b1, dp_w2, dp_b2):
    f32 = np.float32
    xf = np.ascontiguousarray(x.reshape(T_FULL, D).astype(f32, copy=False))
    dp1t = np.ascontiguousarray(dp_w1.reshape(KD, P, H1).transpose(1, 0, 2))
    dp2t = np.ascontiguousarray(dp_w2[:, 0].reshape(MH1, P).T)
    gwt = np.ascontiguousarray(gate_w.reshape(KD, P, E).transpose(1, 0, 2))
    gbt = np.ascontiguousarray(np.broadcast_to(gate_b, (P, E)))
    dpb1 = np.ascontiguousarray(dp_b1.reshape(MH1, P).T)
    dpb2 = np.full((P, 1), dp_b2[0], dtype=f32)
    shared = {"dp1t": dp1t, "dp2t": dp2t, "gwt": gwt, "gbt": gbt,
              "dpb1": dpb1, "dpb2": dpb2}
    in_maps = []
    for c in range(N_CORES):
        xc = xf[c * TC:(c + 1) * TC]
        xt = np.ascontiguousarray(xc.T.reshape(KD, P, TC).transpose(1, 0, 2))
        in_maps.append({"xt": xt, **shared})
    return in_maps, xf


def dispatch(dense_w):
    """Group tokens by assigned expert, split each expert across 2 cores."""
    halves = []
    for e in range(E):
        idx = np.nonzero(dense_w[:, e] > 0)[0]
        h = (len(idx) + 1) // 2
        halves.append(idx[:h])
        halves.append(idx[h:])
    cmax = max((len(h) for h in halves), default=1)
    C = max(((cmax + P - 1) // P) * P, P)
    return halves, C


def prep_ffn_in_maps(xf, halves, C, ew1, eb1, ew2):
    w1ts = [np.ascontiguousarray(
        ew1[e].reshape(KD, P, KH, P).transpose(1, 2, 0, 3)) for e in range(E)]
    w2ts = [np.ascontiguousarray(
        ew2[e].reshape(KH, P, ND, P).transpose(2, 1, 0, 3)) for e in range(E)]
    eb1ts = [np.ascontiguousarray(eb1[e].reshape(KH, P).T) for e in range(E)]
    in_maps = []
    for c in range(N_CORES):
        e = c // 2
        tok = halves[c]
        xp = np.zeros((C, D), dtype=np.float32)
        xp[:len(tok)] = xf[tok]
        xg = np.ascontiguousarray(xp.T.reshape(KD, P, C).transpose(1, 0, 2))
        in_maps.append({
            "xg": xg, "w1g": w1ts[e], "w2g": w2ts[e], "eb1g": eb1ts[e],
        })
    return in_maps


def remap_names(in_maps, names):
    return [{names[k]: v for k, v in m.items()} for m in in_maps]


_BUILT = {}


def _get(key, builder):
    if key not in _BUILT:
        _BUILT[key] = builder()
    return _BUILT[key]


def kernel(x, gate_w, gate_b, dp_w1, dp_b1, dp_w2, dp_b2, ew1, eb1, ew2, eb2,
           trace=False):
    from concourse.bass_utils import run_bass_kernel_spmd

    cores = list(range(N_CORES))

    # ---- phase 1: routing ----
    nc1, names1 = _get("routing", build_routing_nc)
    in1, xf = prep_routing_in_maps(
        x, gate_w, gate_b, dp_w1, dp_b1, dp_w2, dp_b2
    )
    res1 = run_bass_kernel_spmd(nc1, remap_names(in1, names1), cores,
                                trace=trace)
    dense_w = np.concatenate(
        [r[names1["dw"]].transpose(1, 0, 2).reshape(TC, E)
         for r in res1.results]
    )  # [T_FULL, E]

    # ---- host all-to-all dispatch ----
    halves, C = dispatch(dense_w)

    # ---- phase 2: expert FFNs ----
    nc2, names2 = _get(("ffn", C), lambda: build_ffn_nc(C))
    in2 = prep_ffn_in_maps(xf, halves, C, ew1, eb1, ew2)
    res2 = run_bass_kernel_spmd(nc2, remap_names(in2, names2), cores,
                                trace=trace)

    # ---- host combine (scatter-add with routing weights) ----
    out = np.zeros((T_FULL, D), dtype=np.float64)
    for c in range(N_CORES):
        e = c // 2
        tok = halves[c]
        if len(tok) == 0:
            continue
        yt = res2.results[c][names2["yt"]]            # [P, ND, C]
        yg = yt.transpose(2, 1, 0).reshape(C, D)[:len(tok)]
        out[tok] += dense_w[tok, e, None].astype(np.float64) * yg
    if np.any(eb2):
        out += dense_w.astype(np.float64) @ eb2.astype(np.float64)
    out = out.astype(np.float32).reshape(4, 2048, D)
    if trace:
        return out, (res1, res2)
    return out


# revision 13
# speedup vs baseline: 2.1285x; 1.2854x over previous
"""Trainium2 Bass kernel for nn_ConditionalMoELayer (expert-parallel, sparse).

Two-phase design following the expert-parallel sharding hint:

Phase 1 (routing nets, data-parallel): the 8192 tokens are split across the
8 cores (1024 each). Each core runs the difficulty net and the gate on its
tokens in fp32r and ships the raw difficulty logit z and gate logits back.
The host finishes the (tiny, [8192 x 4]) discrete routing math in fp64:
k(z) via monotone softplus-inverted thresholds, top-k by rank, masked
softmax -> dense_w. Tokens whose z or logit-gap sits within 1e-3 of a
decision boundary (a handful) are recomputed exactly in fp64 so the
discrete decisions match the fp32 reference despite fp32r matmul noise.

Host dispatch (the "all-to-all"): tokens are grouped by assigned expert
(dense_w[t,e] > 0); each expert's token set is split over 2 cores
(4 experts x 2 = 8 cores), zero-padded to a common static capacity C.

Phase 2 (expert FFN, expert-parallel): each core holds ONE expert's weights
and computes y = relu(x @ W1 + b1) @ W2 for its gathered tokens in fp32r
(single-pass PE, full rate). All matmuls have 512-wide moving operands so
the 4-byte weight loads stay hidden; activations ride the gpsimd DMA queue
so they never wait behind weight loads. Host scales rows by dense_w and
scatter-adds into the output (token sets within one expert are disjoint).

Only ~40% of token-expert pairs are active, so phase 2 does ~2.5x fewer
FLOPs than the dense equivalent.
"""

import numpy as np

P = 128          # partitions
D = 1024         # d_model
H = 2048         # expert hidden
E = 4            # experts
H1 = 512         # difficulty-net hidden
T_FULL = 8192    # total tokens
N_CORES = 8
TC = T_FULL // N_CORES   # tokens per core in phase 1

KD = D // P      # 8   k-subtiles over D
KH = H // P      # 16  k-subtiles over H
MH1 = H1 // P    # 4   m-tiles of difficulty hidden
NT = TC // P     # 8   token tiles per core (phase 1)
ND = D // P      # 8   d-tiles (phase 2 output)
TCH = 512        # token chunk for matmul moving operand
NTCH = TC // TCH # 2

TH_LO = 0.5
TH_HI = 2.0
MIN_E = 1
RISK_MARGIN = 1e-3   # fp32r logit noise is ~3e-4 worst case; 3x cushion


def _ap_name(t):
    return t.tensor.name if hasattr(t, "tensor") else t.name


def build_routing_nc():
    """Phase-1 module: difficulty-net z and gate logits (transposed out)."""
    import concourse.mybir as mybir
    import concourse.tile as tile
    from concourse import bacc
    from contextlib import ExitStack

    f32 = mybir.dt.float32
    f32r = mybir.dt.float32r
    AF = mybir.ActivationFunctionType

    nc = bacc.Bacc(None, target_bir_lowering=False, debug=False)

    with tile.TileContext(nc) as tc:
        with ExitStack() as ctx:
            dram = ctx.enter_context(tc.tile_pool(name="dram", bufs=1, space="DRAM"))
            xt_d = dram.tile([P, KD, TC], f32r, kind="ExternalInput", name="xt")
            dp1t_d = dram.tile([P, KD, H1], f32r, kind="ExternalInput", name="dp1t")
            dp2t_d = dram.tile([P, MH1], f32r, kind="ExternalInput", name="dp2t")
            gwt_d = dram.tile([P, KD, E], f32r, kind="ExternalInput", name="gwt")
            dpb1_d = dram.tile([P, MH1], f32, kind="ExternalInput", name="dpb1")
            z_d = dram.tile([1, TC], f32, kind="ExternalOutput", name="zt")
            lt_d = dram.tile([E, TC], f32, kind="ExternalOutput", name="lt")

            const = ctx.enter_context(tc.tile_pool(name="const", bufs=1))
            xtp = ctx.enter_context(tc.tile_pool(name="xtp", bufs=1))
            h1p = ctx.enter_context(tc.tile_pool(name="h1p", bufs=2))
            outp = ctx.enter_context(tc.tile_pool(name="outp", bufs=1))
            psb = ctx.enter_context(tc.tile_pool(name="psb", bufs=4, space="PSUM"))
            pslp = ctx.enter_context(tc.tile_pool(name="pslp", bufs=2, space="PSUM"))
            psep = ctx.enter_context(tc.tile_pool(name="psep", bufs=2, space="PSUM"))

            xt_sb = xtp.tile([P, KD, TC], f32r, tag="xt")
            dp1t_sb = const.tile([P, KD, H1], f32r, tag="dp1t")
            dp2t_sb = const.tile([P, MH1], f32r, tag="dp2t")
            gwt_sb = const.tile([P, KD, E], f32r, tag="gwt")
            dpb1_sb = const.tile([P, MH1], f32, tag="dpb1")
            z_sb = outp.tile([1, TC], f32, tag="zsb")
            lt_sb = outp.tile([E, TC], f32, tag="ltsb")

            nc.sync.dma_start(gwt_sb[:], gwt_d[:])
            nc.sync.dma_start(dpb1_sb[:], dpb1_d[:])
            nc.sync.dma_start(dp2t_sb[:], dp2t_d[:])
            for mt in range(MH1):
                nc.sync.dma_start(
                    dp1t_sb[:, :, mt * P:(mt + 1) * P],
                    dp1t_d[:, :, mt * P:(mt + 1) * P],
                )
            # x rides the gpsimd (SWDGE) queue, split per chunk
            for t2 in range(NTCH):
                nc.gpsimd.dma_start(
                    xt_sb[:, :, t2 * TCH:(t2 + 1) * TCH],
                    xt_d[:, :, t2 * TCH:(t2 + 1) * TCH],
                )

            for t2 in range(NTCH):
                tsl = slice(t2 * TCH, (t2 + 1) * TCH)
                # gate logits, transposed: psum[E, cw] = gw.T @ x
                psl = pslp.tile([E, TCH], f32, tag="psl")
                for ko in range(KD):
                    nc.tensor.matmul(
                        psl,
                        gwt_sb[:, ko, :],
                        xt_sb[:, ko, tsl],
                        start=(ko == 0),
                        stop=(ko == KD - 1),
                    )
                nc.scalar.activation(lt_sb[:, tsl], psl, AF.Identity)
                # difficulty hidden: h1T[mt] = W1[:,mt].T @ x, relu
                h1T = h1p.tile([P, MH1, TCH], f32r, tag="h1T")
                for mt in range(MH1):
                    ps = psb.tile([P, TCH], f32, tag="psb")
                    for ko in range(KD):
                        nc.tensor.matmul(
                            ps,
                            dp1t_sb[:, ko, mt * P:(mt + 1) * P],
                            xt_sb[:, ko, tsl],
                            start=(ko == 0),
                            stop=(ko == KD - 1),
                        )
                    nc.scalar.activation(
                        h1T[:, mt, :], ps, AF.Relu, bias=dpb1_sb[:, mt:mt + 1]
                    )
                # z (pre-softplus difficulty logit), transposed: [1, cw]
                pse = psep.tile([1, TCH], f32, tag="pse")
                for ko in range(MH1):
                    nc.tensor.matmul(
                        pse,
                        dp2t_sb[:, ko:ko + 1],
                        h1T[:, ko, :],
                        start=(ko == 0),
                        stop=(ko == MH1 - 1),
                    )
                nc.scalar.activation(z_sb[:, tsl], pse, AF.Identity)

            nc.sync.dma_start(z_d[:], z_sb[:])
            nc.sync.dma_start(lt_d[:], lt_sb[:])

    nc.compile()
    names = {k: _ap_name(v) for k, v in {
        "xt": xt_d, "dp1t": dp1t_d, "dp2t": dp2t_d, "gwt": gwt_d,
        "dpb1": dpb1_d, "zt": z_d, "lt": lt_d,
    }.items()}
    return nc, names


def build_ffn_nc(C):
    """Phase-2 module: one expert FFN over C gathered tokens per core.

    All matmuls use up-to-512-wide moving operands (tokens): mm1 computes
    hid^T = W1^T x^T, mm2 computes y^T = W2^T hid^T (weights stationary).
    """
    import concourse.mybir as mybir
    import concourse.tile as tile
    from concourse import bacc
    from contextlib import ExitStack

    f32 = mybir.dt.float32
    f32r = mybir.dt.float32r
    AF = mybir.ActivationFunctionType

    chunks = []
    c0 = 0
    while c0 < C:
        cw = min(TCH, C - c0)
        chunks.append((c0, cw))
        c0 += cw

    nc = bacc.Bacc(None, target_bir_lowering=False, debug=False)

    with tile.TileContext(nc) as tc:
        with ExitStack() as ctx:
            dram = ctx.enter_context(tc.tile_pool(name="dram", bufs=1, space="DRAM"))
            xg_d = dram.tile([P, KD, C], f32r, kind="ExternalInput", name="xg")
            w1_d = dram.tile([P, KH, KD, P], f32r, kind="ExternalInput", name="w1g")
            w2_d = dram.tile([ND, P, KH, P], f32r, kind="ExternalInput", name="w2g")
            eb1_d = dram.tile([P, KH], f32, kind="ExternalInput", name="eb1g")
            yt_d = dram.tile([P, ND, C], f32, kind="ExternalOutput", name="yt")

            const = ctx.enter_context(tc.tile_pool(name="const", bufs=1))
            xgp = ctx.enter_context(tc.tile_pool(name="xgp", bufs=2))
            hidp = ctx.enter_context(tc.tile_pool(name="hidp", bufs=1))
            w2p = ctx.enter_context(tc.tile_pool(name="w2p", bufs=3))
            ytp = ctx.enter_context(tc.tile_pool(name="ytp", bufs=2))
            psb = ctx.enter_context(tc.tile_pool(name="psb", bufs=3, space="PSUM"))
            ps2p = ctx.enter_context(tc.tile_pool(name="ps2p", bufs=3, space="PSUM"))

            w1_sb = const.tile([P, KH, KD, P], f32r, tag="w1")
            eb1_sb = const.tile([P, KH], f32, tag="eb1")
            nc.sync.dma_start(eb1_sb[:], eb1_d[:])
            # W1 resident (64KB/partition), loaded per-ht so matmuls can
            # start before the whole 8MB lands
            for ht in range(KH):
                nc.sync.dma_start(w1_sb[:, ht], w1_d[:, ht])

            for (c0, cw) in chunks:
                csl = slice(c0, c0 + cw)
                xg_sb = xgp.tile([P, KD, TCH], f32r, tag="xg")
                nc.gpsimd.dma_start(xg_sb[:, :, :cw], xg_d[:, :, csl])
                hidT = hidp.tile([P, KH, TCH], f32r, tag="hid")
                for ht in range(KH):
                    ps = psb.tile([P, TCH], f32, tag="psb")
                    for ko in range(KD):
                        nc.tensor.matmul(
                            ps[:, :cw],
                            w1_sb[:, ht, ko, :],
                            xg_sb[:, ko, :cw],
                            start=(ko == 0),
                            stop=(ko == KD - 1),
                        )
                    nc.scalar.activation(
                        hidT[:, ht, :cw], ps[:, :cw], AF.Relu,
                        bias=eb1_sb[:, ht:ht + 1],
                    )
                yt_sb = ytp.tile([P, ND, TCH], f32, tag="yt")
                for dt in range(ND):
                    w2s = w2p.tile([P, KH, P], f32r, tag="w2")
                    nc.sync.dma_start(w2s[:], w2_d[dt])
                    ps2 = ps2p.tile([P, TCH], f32, tag="ps2")
                    for ko in range(KH):
                        nc.tensor.matmul(
                            ps2[:, :cw],
                            w2s[:, ko, :],
                            hidT[:, ko, :cw],
                            start=(ko == 0),
                            stop=(ko == KH - 1),
                        )
                    nc.vector.tensor_copy(yt_sb[:, dt, :cw], ps2[:, :cw])
                nc.gpsimd.dma_start(yt_d[:, :, csl], yt_sb[:, :, :cw])

    nc.compile()
    names = {k: _ap_name(v) for k, v in {
        "xg": xg_d, "w1g": w1_d, "w2g": w2_d, "eb1g": eb1_d, "yt": yt_d,
    }.items()}
    return nc, names


def prep_routing_in_maps(x, dp_w1, dp_b1, dp_w2, gate_w):
    f32 = np.float32
    xf = np.ascontiguousarray(x.reshape(T_FULL, D).astype(f32, copy=False))
    dp1t = np.ascontiguousarray(dp_w1.reshape(KD, P, H1).transpose(1, 0, 2))
    dp2t = np.ascontiguousarray(dp_w2[:, 0].reshape(MH1, P).T)
    gwt = np.ascontiguousarray(gate_w.reshape(KD, P, E).transpose(1, 0, 2))
    dpb1 = np.ascontiguousarray(dp_b1.reshape(MH1, P).T)
    shared = {"dp1t": dp1t, "dp2t": dp2t, "gwt": gwt, "dpb1": dpb1}
    in_maps = []
    for c in range(N_CORES):
        xc = xf[c * TC:(c + 1) * TC]
        xt = np.ascontiguousarray(xc.T.reshape(KD, P, TC).transpose(1, 0, 2))
        in_maps.append({"xt": xt, **shared})
    return in_maps, xf


def finish_routing(z, logits, xf, gate_w, gate_b, dp_w1, dp_b1, dp_w2, dp_b2):
    """Host fp64 finisher for the discrete routing decisions.

    z, logits carry fp32r matmul noise (~3e-4 worst case). Any token whose
    decision sits within RISK_MARGIN of a boundary is recomputed exactly in
    fp64, so k / top-k selections match the fp32 reference.
    """
    T = len(z)
    z = z.astype(np.float64) + float(dp_b2[0])
    logits = logits.astype(np.float64) + gate_b.astype(np.float64)[None, :]
    xf64 = None

    # exact recompute helpers
    def exact_z(tok):
        h = np.maximum(xf64[tok] @ dp_w1.astype(np.float64)
                       + dp_b1.astype(np.float64), 0)
        return h @ dp_w2.astype(np.float64)[:, 0] + float(dp_b2[0])

    def exact_logits(tok):
        return xf64[tok] @ gate_w.astype(np.float64) \
            + gate_b.astype(np.float64)[None, :]

    step = (TH_HI - TH_LO) / (2 * (E - MIN_E))
    zts = [np.log(np.expm1(TH_LO + (2 * i - 1) * step)) for i in (1, 2, 3)]

    risk_z = np.zeros(T, bool)
    for zt in zts:
        risk_z |= np.abs(z - zt) < RISK_MARGIN
    if risk_z.any():
        xf64 = xf.astype(np.float64)
        tok = np.nonzero(risk_z)[0]
        z[tok] = exact_z(tok)

    sl = np.sort(logits, axis=1)
    gaps = np.diff(sl, axis=1)
    risk_l = (gaps < RISK_MARGIN).any(axis=1)
    if risk_l.any():
        if xf64 is None:
            xf64 = xf.astype(np.float64)
        tok = np.nonzero(risk_l)[0]
        logits[tok] = exact_logits(tok)

    # k = round(1 + 3*clip((softplus(z)-0.5)/1.5, 0, 1)) via monotone
    # z-thresholds; middle threshold strict (RNE rounds kraw=2.5 down to 2)
    k = 1 + (z >= zts[0]).astype(np.int64) + (z > zts[1]) + (z >= zts[2])

    order = np.argsort(-logits, axis=1, kind="stable")
    rank = np.empty_like(order)
    rank[np.arange(T)[:, None], order] = np.arange(E)[None, :]
    sel = rank < k[:, None]

    m = logits.max(axis=1, keepdims=True)
    ex = np.where(sel, np.exp(logits - m), 0.0)
    dense_w = ex / ex.sum(axis=1, keepdims=True)
    return dense_w


def dispatch(dense_w):
    """Group tokens by assigned expert, split each expert across 2 cores."""
    halves = []
    for e in range(E):
        idx = np.nonzero(dense_w[:, e] > 0)[0]
        h = (len(idx) + 1) // 2
        halves.append(idx[:h])
        halves.append(idx[h:])
    cmax = max((len(h) for h in halves), default=1)
    C = max(((cmax + P - 1) // P) * P, P)
    return halves, C


def prep_ffn_in_maps(xf, halves, C, ew1, eb1, ew2):
    w1ts = [np.ascontiguousarray(
        ew1[e].reshape(KD, P, KH, P).transpose(1, 2, 0, 3)) for e in range(E)]
    w2ts = [np.ascontiguousarray(
        ew2[e].reshape(KH, P, ND, P).transpose(2, 1, 0, 3)) for e in range(E)]
    eb1ts = [np.ascontiguousarray(eb1[e].reshape(KH, P).T) for e in range(E)]
    in_maps = []
    for c in range(N_CORES):
        e = c // 2
        tok = halves[c]
        xp = np.zeros((C, D), dtype=np.float32)
        xp[:len(tok)] = xf[tok]
        xg = np.ascontiguousarray(xp.T.reshape(KD, P, C).transpose(1, 0, 2))
        in_maps.append({
            "xg": xg, "w1g": w1ts[e], "w2g": w2ts[e], "eb1g": eb1ts[e],
        })
    return in_maps


def remap_names(in_maps, names):
    return [{names[k]: v for k, v in m.items()} for m in in_maps]


_BUILT = {}


def _get(key, builder):
    if key not in _BUILT:
        _BUILT[key] = builder()
    return _BUILT[key]


def kernel(x, gate_w, gate_b, dp_w1, dp_b1, dp_w2, dp_b2, ew1, eb1, ew2, eb2,
           trace=False):
    from concourse.bass_utils import run_bass_kernel_spmd

    cores = list(range(N_CORES))

    # ---- phase 1: routing nets on device ----
    nc1, names1 = _get("routing", build_routing_nc)
    in1, xf = prep_routing_in_maps(x, dp_w1, dp_b1, dp_w2, gate_w)
    res1 = run_bass_kernel_spmd(nc1, remap_names(in1, names1), cores,
                                trace=trace)
    z = np.concatenate([r[names1["zt"]][0] for r in res1.results])
    logits = np.concatenate([r[names1["lt"]].T for r in res1.results])

    # ---- host: finish routing (fp64 + exact boundary fixups) ----
    dense_w = finish_routing(
        z, logits, xf, gate_w, gate_b, dp_w1, dp_b1, dp_w2, dp_b2
    )
    halves, C = dispatch(dense_w)

    # ---- phase 2: expert FFNs ----
    nc2, names2 = _get(("ffn", C), lambda: build_ffn_nc(C))
    in2 = prep_ffn_in_maps(xf, halves, C, ew1, eb1, ew2)
    res2 = run_bass_kernel_spmd(nc2, remap_names(in2, names2), cores,
                                trace=trace)

    # ---- host combine (scatter-add with routing weights) ----
    out = np.zeros((T_FULL, D), dtype=np.float64)
    for c in range(N_CORES):
        e = c // 2
        tok = halves[c]
        if len(tok) == 0:
            continue
        yt = res2.results[c][names2["yt"]]            # [P, ND, C]
        yg = yt.transpose(2, 1, 0).reshape(C, D)[:len(tok)]
        out[tok] += dense_w[tok, e, None] * yg
    if np.any(eb2):
        out += dense_w @ eb2.astype(np.float64)
    out = out.astype(np.float32).reshape(4, 2048, D)
    if trace:
        return out, (res1, res2)
    return out
